# revision 74
# baseline (speedup 1.0000x reference)
"""Multi-head attention on 8 NeuronCores (Trainium2, Bass/Tile).

Problem: B=2, S=2048, E=1024, H=16, D=64 MHA with int mask, fp32.

Sharding: core c = 4*b + g handles batch b, head group g (4 heads = a
256-wide slice of E).  Wo is row-sharded; the host sums the 4 partials
per batch and adds bo.

Per-core design (all fp16 matmul operands, fp32 PSUM accumulation),
built as one software-pipelined stream of 128 "slots" (qc, head, ks):

  scores   PE: scores_T [k128, q1024] = khT.T @ qhT per slot into a
           double-buffered 2-bank PSUM tile.
  exp      ACT: exp(0.125 x) -> p fp16; this is the ONLY Activation-
           engine work (its 1.04us/slot is the steady-state pace).
  mask     DVE: p *= mask (fp16 2x mode).  The mask is DMA'd as fp8
           (halves the DMA-bound startup window) and upconverted to a
           single-qc fp16 buffer by the otherwise idle Pool engine.
  ctx      PE: q-partition layout ctx[q128, 64] += p_slice.T @ vh per
           q-subtile (full 128-wide PE output bandwidth, half the cost
           of the d-partition layout) + a 1-column denominator matmul
           against a ones vector; ctx consumption lags production by
           CTX_LAG slots so PE never waits on the DVE queue.
  norm     DVE: reciprocal + per-partition tensor_scalar_mul into
           head-pair slabs ctxn [q, (hh d)], spread 2 ops/slot.
  ctxT     one xbar DMA transpose per (head-pair, qc): ctxn -> [j, q].
  outproj  PE: out[q128, e512] = ctxT.T @ woT; qc0's tiles interleave
           into qc1's slots through the single spare PSUM bank, qc1's
           run in a 6-bank tail pipeline with copies alternating
           between ACT (idle after the last exp) and DVE.

Phase A (projections) has no pools of its own: units are emitted into
the slot stream at deadline-derived positions (unit_plan), borrowing
the ctx pool (preamble) and the spare bank; x chunks stream per
512-column slice so the first exp fires ~15us in, limited by the
serialized DMA device (the first ~45us is DMA-saturated).

PSUM: scores 2x2 banks + ctx 2 + den 1 + spare 1 = 8 banks exactly.
"""

import os
import sys

sys.path.insert(0, "/opt/trn_rl_repo")

import ml_dtypes
import numpy as np

import concourse.mybir as mybir
import concourse.tile as tile
from concourse import bacc
from concourse import bass_utils

B, S, E, H = 2, 2048, 1024, 16
D = E // H              # 64
G = 4                   # head groups (cores per batch)
HL = H // G             # 4 local heads per core
J = HL * D              # 256 local j width
P = 128
KT = E // P             # 8 k-tiles for projections
ST = S // P             # 16 s-tiles / ks-tiles
NQ = 1024               # q-chunk width for attention
QC = S // NQ            # 2 q chunks
SC = 4                  # projection s-chunks of 512
QS = NQ // P            # 8 q-subtiles per chunk

F32 = mybir.dt.float32
F16 = mybir.dt.float16
F8 = mybir.dt.float8e4

# Exposed for test.py / bench.py.
LAST_RESULTS = None
LAST_NC = None


def _f16(x: np.ndarray) -> np.ndarray:
    return np.ascontiguousarray(x, dtype=np.float32).astype(np.float16)


def _build_program(use_bias_qk: bool, use_bias_v: bool):
    nc = bacc.Bacc("TRN2", target_bir_lowering=False, debug=False, num_devices=8)

    xqT = nc.dram_tensor("xqT", [E, S], F16, kind="ExternalInput")
    xkT = nc.dram_tensor("xkT", [E, S], F16, kind="ExternalInput")
    xvT = nc.dram_tensor("xvT", [E, S], F16, kind="ExternalInput")
    maskT = nc.dram_tensor("maskT", [S, S], F8, kind="ExternalInput")
    wqT = nc.dram_tensor("wqT", [E, J], F16, kind="ExternalInput")
    wkT = nc.dram_tensor("wkT", [E, J], F16, kind="ExternalInput")
    wvT = nc.dram_tensor("wvT", [E, J], F16, kind="ExternalInput")
    woT = nc.dram_tensor("woT", [J, E], F16, kind="ExternalInput")
    bqk = nc.dram_tensor("bqk", [2, J], F32, kind="ExternalInput")
    bv = nc.dram_tensor("bv", [1, J], F32, kind="ExternalInput")
    out = nc.dram_tensor("out", [S, E], F16, kind="ExternalOutput")

    Exp = mybir.ActivationFunctionType.Exp

    with tile.TileContext(nc) as tc:
        with (
            tc.tile_pool(name="consts", bufs=1) as consts,
            tc.tile_pool(name="persist", bufs=1) as persist,
            tc.tile_pool(name="xs", bufs=2) as xs,
            tc.tile_pool(name="maskp", bufs=1) as maskp,
            tc.tile_pool(name="pwork", bufs=16) as pwork,
            tc.tile_pool(name="osb", bufs=4) as osb,
            tc.tile_pool(name="small", bufs=2) as small,
        ):
            # ---- constants / weights (DMAs emitted at point of need) ----
            wq_sb = consts.tile([P, KT, J], F16, tag="wq")
            wk_sb = consts.tile([P, KT, J], F16, tag="wk")
            wv_sb = consts.tile([P, KT, J], F16, tag="wv")
            wo_sb = consts.tile([P, 2, E], F16, tag="wo")
            ones = consts.tile([P, 1], F16, tag="ones")
            nc.gpsimd.memset(ones[:], 1.0)
            # dummy exp to pull the ACT function-table load off the first
            # real exp's critical path (runs concurrently with input DMAs)
            scratch = consts.tile([1, 2], F16, tag="scratch")
            nc.gpsimd.memset(scratch[:], 0.0)
            nc.scalar.activation(scratch[:], scratch[:], Exp)
            nc.sync.dma_start(wq_sb[:], wqT.rearrange("(kt p) j -> p kt j", p=P))

            if use_bias_qk:
                bqk_sb = consts.tile([P, 2, 2], F32, tag="bqk")  # [p, qk, hp]
                nc.sync.dma_start(
                    bqk_sb[:], bqk.rearrange("qk (hp p) -> p qk hp", p=P)
                )
            if use_bias_v:
                bv_row = consts.tile([1, J], F32, tag="bvr")
                nc.sync.dma_start(bv_row[:], bv[:, :])
                bv_bc = consts.tile([P, J], F32, tag="bvb")
                nc.gpsimd.partition_broadcast(bv_bc[:], bv_row[:])

            # ---- persistent activations ----
            qh = persist.tile([P, 2, S], F16, tag="qh")
            kh = persist.tile([P, 2, S], F16, tag="kh")
            vh = persist.tile([P, ST, HL, D], F16, tag="vh")
            ctxn = [persist.tile([P, QS, P], F16, tag=f"ctxn{hp}", name=f"ctxn{hp}")
                    for hp in range(2)]
            ctxT = persist.tile([P, 2, S], F16, tag="ctxT")
            mask_sb = maskp.tile([P, ST, NQ], F16, tag="mask")

            # ---- phase B/C pools (all of PSUM: 4+2+1+1 banks) ----
            # Phase A borrows these rings: the preamble pipelines through
            # ctxp (2 banks, idle until the first ctx accumulation), the
            # slot-interleaved units go 1-deep through osp0 (idle until the
            # qc0 output projection starts at slot ~66).
            scorep_cm = tc.tile_pool(name="scorep", bufs=2, space="PSUM")
            scorep = scorep_cm.__enter__()
            ctxp_cm = tc.tile_pool(name="ctxp", bufs=2, space="PSUM")
            ctxp = ctxp_cm.__enter__()
            denp_cm = tc.tile_pool(name="denp", bufs=1, space="PSUM")
            denp = denp_cm.__enter__()
            osp0_cm = tc.tile_pool(name="osp0", bufs=1, space="PSUM")
            osp0 = osp0_cm.__enter__()

            xtiles = {}
            nproj = [0]

            def x_chunk(which, sc):
                key = (which, sc)
                if key in xtiles:
                    return xtiles[key]
                src = {"q": xqT, "k": xkT, "v": xvT}[which]
                bufs = 4 if which == "k" else 2
                t = xs.tile([P, KT, 512], F16, tag=f"x{which}",
                            name=f"x{which}{sc}", bufs=bufs)
                nc.sync.dma_start(
                    t[:],
                    src[:, sc * 512:(sc + 1) * 512].rearrange(
                        "(kt p) s -> p kt s", p=P),
                )
                xtiles[key] = t
                return t

            def proj_psum(pool, name):
                nproj[0] += 1
                tag = "ctx" if pool is ctxp else "ops"
                return pool.tile([P, 512], F32, tag=tag,
                                 name=f"{name}_{nproj[0]}")

            def qk_proj_unit(pool, which, hp, sc, half=None, part=None,
                             xt=None):
                """One [j128, s512] projection tile for q or k.

                half=0/1 emits only 256 s-columns; part=(s0, w) an arbitrary
                column span (finer interleaving so early k tiles don't
                starve the attention stream).
                """
                if xt is None:
                    xt = x_chunk(which, sc)
                w_sb = wq_sb if which == "q" else wk_sb
                dst = qh if which == "q" else kh
                if part is not None:
                    s0, w = part
                elif half is None:
                    s0, w = 0, 512
                else:
                    s0, w = half * 256, 256
                acc = proj_psum(pool, f"p{which}{hp}_{sc}_{half}")
                for kt in range(KT):
                    nc.tensor.matmul(
                        acc[:, 0:w],
                        w_sb[:, kt, hp * P:(hp + 1) * P],
                        xt[:, kt, s0:s0 + w],
                        start=(kt == 0), stop=(kt == KT - 1),
                    )
                dview = dst[:, hp, sc * 512 + s0:sc * 512 + s0 + w]
                if use_bias_qk:
                    qki = 0 if which == "q" else 1
                    nc.vector.tensor_scalar_add(
                        dview, acc[:, 0:w], bqk_sb[:, qki, hp:hp + 1])
                else:
                    nc.vector.tensor_copy(dview, acc[:, 0:w])

            def v_proj_unit(pool, st):
                """vh tile for s-tile st: [s128, (h d)=256]."""
                sc = st // 4
                xt = x_chunk("v", sc)
                acc = proj_psum(pool, f"pv{st}")
                for kt in range(KT):
                    nc.tensor.matmul(
                        acc[:, 0:J],
                        xt[:, kt, (st % 4) * P:(st % 4 + 1) * P],
                        wv_sb[:, kt, :],
                        start=(kt == 0), stop=(kt == KT - 1),
                    )
                src3 = acc[:, 0:J].rearrange("p (h d) -> p h d", h=HL)
                if use_bias_v:
                    nc.vector.tensor_add(
                        vh[:, st, :, :], src3,
                        bv_bc[:].rearrange("p (h d) -> p h d", h=HL))
                else:
                    nc.vector.tensor_copy(vh[:, st, :, :], src3)

            def mask_chunk(qc, kg):
                m8 = maskp.tile([P, 4, NQ], F8, tag="m8", bufs=2,
                                name=f"m8_{qc}_{kg}")
                nc.sync.dma_start(
                    m8[:],
                    maskT[kg * 512:(kg + 1) * 512,
                          qc * NQ:(qc + 1) * NQ].rearrange(
                        "(kt p) nq -> p kt nq", p=P),
                )
                for kk in range(4):
                    nc.gpsimd.tensor_copy(
                        mask_sb[:, kg * 4 + kk, :], m8[:, kk, :])

            # Preamble: enough of phase A to start the attention stream.
            # DMA issue order is the critical path here (the DMA device is
            # serialized): wq, xq0, wk, xk0, xq1, mask00, wv, xv0.
            qk_proj_unit(ctxp, "q", 0, 0)
            qk_proj_unit(ctxp, "q", 0, 1)
            nc.sync.dma_start(wk_sb[:], wkT.rearrange("(kt p) j -> p kt j", p=P))
            qk_proj_unit(ctxp, "k", 0, 0, half=0)
            qk_proj_unit(ctxp, "k", 0, 0, half=1)
            x_chunk("k", 1)
            mask_chunk(0, 0)
            nc.sync.dma_start(wv_sb[:], wvT.rearrange("(kt p) j -> p kt j", p=P))
            v_proj_unit(ctxp, 0)
            v_proj_unit(ctxp, 1)

            # Remaining phase A work, in need-by order; one unit per slot
            # (pumped through the osp0 bank, 1-deep).  k(hp0) tiles are
            # emitted as 256-column halves so the h0 ks-stream never waits;
            # xk stays fully resident (bufs=4) so late k(hp1) units don't
            # deadlock against chunk-buffer rotation.
            def K(hp, sc, half):
                return lambda: qk_proj_unit(osp0, "k", hp, sc, half)

            def Q(hp, sc):
                return lambda: qk_proj_unit(osp0, "q", hp, sc)

            def V(st):
                return lambda: v_proj_unit(osp0, st)

            def M(qc, kg):
                return lambda: mask_chunk(qc, kg)

            def WO():
                return lambda: nc.sync.dma_start(
                    wo_sb[:], woT.rearrange("(hp p) e -> p hp e", p=P))

            def K(hp, sc, half):
                return lambda: qk_proj_unit(osp0, "k", hp, sc, half)

            def Q(hp, sc):
                return lambda: qk_proj_unit(osp0, "q", hp, sc)

            def V(st):
                return lambda: v_proj_unit(osp0, st)

            def M(qc, kg):
                return lambda: mask_chunk(qc, kg)

            # Phase-A units pinned to emission slots.  Constraints:
            #  - mask chunk (0,g) before slot 4g (mul reads it);
            #  - mask chunk (1,g) after slot 52+4g (qc0 h3 still reads the
            #    single-buffered rows) and before 64+4g;
            #  - k(0,sc,h) before scores slot 4sc+2h; v(st) before the ctx
            #    pop at slot st+CTX_LAG; hp1/q tiles before slot 32/64.
            unit_plan = {
                0: [M(0, 1), K(0, 1, 0)], 1: [K(0, 1, 1), V(2)],
                2: [M(0, 2), V(3)], 3: [K(0, 2, 0), V(4)],
                4: [K(0, 2, 1), V(5)], 5: [M(0, 3), V(6)],
                6: [K(0, 3, 0), V(7)], 7: [K(0, 3, 1), V(8)],
                8: [V(9)], 9: [V(10)], 10: [V(11)], 11: [V(12)],
                12: [V(13)], 13: [V(14)], 14: [V(15)],
                15: [K(1, 0, 0)], 16: [K(1, 0, 1)],
                17: [K(1, 1, 0)], 18: [K(1, 1, 1)],
                19: [Q(1, 0)], 20: [Q(1, 1)],
                21: [K(1, 2, 0)], 22: [K(1, 2, 1)],
                23: [K(1, 3, 0)], 24: [K(1, 3, 1)],
                26: [Q(0, 2)], 28: [Q(0, 3)], 30: [Q(1, 2)], 32: [Q(1, 3)],
                34: [WO()],
                52: [M(1, 0)], 56: [M(1, 1)], 60: [M(1, 2)], 64: [M(1, 3)],
            }

            def pump_proj(i=None):
                for fn in unit_plan.pop(i, []):
                    fn()

            den_t = denp.tile([P, 512], F32, tag="den")

            outq = []

            def outproj_task(pool, qc, qt, eh):
                ops = pool.tile([P, 512], F32, tag="ops",
                                name=f"ops{qc}_{qt}_{eh}")
                for hp in range(2):
                    nc.tensor.matmul(
                        ops[:],
                        ctxT[:, hp, qc * NQ + qt * P:qc * NQ + (qt + 1) * P],
                        wo_sb[:, hp, eh * 512:(eh + 1) * 512],
                        start=(hp == 0), stop=(hp == 1),
                    )
                o_sb = osb.tile([P, 512], F16, tag="o", bufs=4)
                nc.vector.tensor_copy(o_sb[:], ops[:])
                nc.sync.dma_start(
                    out[(qc * QS + qt) * P:(qc * QS + qt + 1) * P,
                        eh * 512:(eh + 1) * 512],
                    o_sb[:],
                )

            ctx_tiles = {}

            def emit_ctx_den(qc, h, ks, p_t, chase=False):
                """ctx/den accumulation for one (head, ks).  With chase=True
                (the very last slot) the per-qsub reciprocal+normalize chase
                the accumulation so the output tail starts immediately."""
                hp, hh = h // 2, h % 2
                if (qc, h) not in ctx_tiles:
                    ctx_tiles[(qc, h)] = ctxp.tile(
                        [P, 512], F32, tag="ctx", name=f"ctx{qc}_{h}")
                ct3 = ctx_tiles[(qc, h)][:].rearrange(
                    "p (qs d) -> p qs d", qs=QS)
                first = (ks == 0)
                last = (ks == ST - 1)
                for qs in range(QS):
                    st_ = p_t[:, qs * P:(qs + 1) * P]
                    nc.tensor.matmul(
                        ct3[:, qs, :], st_, vh[:, ks, h, :],
                        start=(first and qs == 0), stop=(last and qs == QS - 1),
                    )
                    nc.tensor.matmul(
                        den_t[:, qs:qs + 1], st_, ones[:],
                        start=(first and qs == 0), stop=(last and qs == QS - 1),
                    )
                if last:
                    denr = small.tile([P, QS], F32, tag="denr8")
                    nc.vector.reciprocal(denr[:], den_t[:, 0:QS])

                    def mk(qs, ct3=ct3, denr=denr, hp=hp, hh=hh):
                        def emit():
                            nc.vector.tensor_scalar_mul(
                                ctxn[hp][:, qs, hh * D:(hh + 1) * D],
                                ct3[:, qs, :], denr[:, qs:qs + 1])
                        return emit
                    if chase:
                        for qs in range(QS):
                            mk(qs)()
                    else:
                        for qs in range(QS):
                            norm_q.append(mk(qs))

            pending_tr = []
            norm_q = []

            def emit_transpose(qc, hp):
                nc.sync.dma_start_transpose(
                    ctxT[:, hp, qc * NQ:(qc + 1) * NQ].rearrange(
                        "p (qs q) -> p qs q", q=P),
                    ctxn[hp][:],
                )
                if hp == 1:
                    # ctxT for this qc is complete; queue its output tiles
                    for qt in range(QS):
                        for eh in range(2):
                            outq.append((qc, qt, eh))

            def emit_finalize(qc, h, slot_i):
                hp, hh = h // 2, h % 2
                ctx_tiles.pop((qc, h))
                if hh == 1:
                    # Deferred by 2 slots so the transpose doesn't camp at
                    # the head of the SP queue waiting on the normmuls,
                    # blocking DMAs queued behind it.
                    pending_tr.append((slot_i + 2, qc, hp))

            # ---- the slot stream ----
            # ctx/den consumption lags the scores/exp/mask production by
            # CTX_LAG slots so PE never waits on the DVE mask multiply.
            CTX_LAG = 9
            slots = [(qc, h, ks)
                     for qc in range(QC) for h in range(HL) for ks in range(ST)]
            pending = []
            for i, (qc, h, ks) in enumerate(slots):
                hp, hh = h // 2, h % 2
                sp = scorep.tile([P, NQ], F32, tag="sp", name=f"sp{qc}_{h}_{ks}")
                for n2 in range(2):
                    nc.tensor.matmul(
                        sp[:, n2 * 512:(n2 + 1) * 512],
                        kh[hh * D:(hh + 1) * D, hp, ks * P:(ks + 1) * P],
                        qh[hh * D:(hh + 1) * D, hp,
                           qc * NQ + n2 * 512:qc * NQ + (n2 + 1) * 512],
                        start=True, stop=True,
                    )
                p_t = pwork.tile([P, NQ], F16, tag="pt")
                nc.scalar.activation(p_t[:], sp[:], Exp, scale=0.125)
                nc.vector.tensor_mul(p_t[:], p_t[:], mask_sb[:, ks, :])
                pending.append((qc, h, ks, p_t))
                if len(pending) > CTX_LAG:
                    pv = pending.pop(0)
                    emit_ctx_den(*pv)
                    if pv[2] == ST - 1:
                        emit_finalize(pv[0], pv[1], i)
                for _ in range(2):
                    if norm_q:
                        norm_q.pop(0)()
                while pending_tr and pending_tr[0][0] <= i:
                    while norm_q:
                        norm_q.pop(0)()
                    _, tqc, thp = pending_tr.pop(0)
                    emit_transpose(tqc, thp)
                pump_proj(i)
                if outq and i % 2 == 1:
                    # every other slot: the 1-deep osp0 ring plus DVE queue
                    # latency needs ~2 slots between successive tiles
                    outproj_task(osp0, *outq.pop(0))
            while pending:
                pv = pending.pop(0)
                emit_ctx_den(*pv, chase=not pending)
                if pv[2] == ST - 1:
                    emit_finalize(pv[0], pv[1], len(slots))
            while norm_q:
                norm_q.pop(0)()
            while pending_tr:
                _, tqc, thp = pending_tr.pop(0)
                emit_transpose(tqc, thp)
            for pos in sorted(unit_plan):
                pump_proj(pos)

            osp0_cm.__exit__(None, None, None)
            denp_cm.__exit__(None, None, None)
            ctxp_cm.__exit__(None, None, None)
            scorep_cm.__exit__(None, None, None)

            # ---- qc1 output projection tail ----
            # ACT is idle after the last exp: alternate the PSUM->SBUF
            # copies between ACT and DVE, one merged [128, E] DMA per qtile.
            Copy = mybir.ActivationFunctionType.Copy
            tailp_cm = tc.tile_pool(name="tailp", bufs=6, space="PSUM")
            tailp = tailp_cm.__enter__()
            o2_tiles = {}
            while outq:
                tqc, qt, eh = outq.pop(0)
                if (tqc, qt) not in o2_tiles:
                    o2_tiles[(tqc, qt)] = (osb.tile(
                        [P, E], F16, tag="o2", bufs=4,
                        name=f"o2_{tqc}_{qt}"), [])
                o2, done = o2_tiles[(tqc, qt)]
                ops = tailp.tile([P, 512], F32, tag="tops",
                                 name=f"t{tqc}_{qt}_{eh}")
                for hp_ in range(2):
                    nc.tensor.matmul(
                        ops[:],
                        ctxT[:, hp_, tqc * NQ + qt * P:
                             tqc * NQ + (qt + 1) * P],
                        wo_sb[:, hp_, eh * 512:(eh + 1) * 512],
                        start=(hp_ == 0), stop=(hp_ == 1),
                    )
                if eh == 0:
                    nc.scalar.activation(o2[:, 0:512], ops[:], Copy)
                else:
                    nc.vector.tensor_copy(o2[:, 512:E], ops[:])
                done.append(eh)
                if len(done) == 2:
                    nc.sync.dma_start(
                        out[(tqc * QS + qt) * P:(tqc * QS + qt + 1) * P, :],
                        o2[:])
            tailp_cm.__exit__(None, None, None)

    nc.compile()
    return nc


def kernel(q, k, v, mask, Wq, bq, Wk, bk, Wv, bv, Wo, bo):
    global LAST_RESULTS, LAST_NC
    q = np.asarray(q, np.float32)
    k = np.asarray(k, np.float32)
    v = np.asarray(v, np.float32)
    mask = np.asarray(mask)
    Wq = np.asarray(Wq, np.float32)
    Wk = np.asarray(Wk, np.float32)
    Wv = np.asarray(Wv, np.float32)
    Wo = np.asarray(Wo, np.float32)
    bq = np.asarray(bq, np.float32)
    bk = np.asarray(bk, np.float32)
    bv = np.asarray(bv, np.float32)
    bo = np.asarray(bo, np.float32)

    use_bias_qk = bool(np.any(bq) or np.any(bk))
    use_bias_v = bool(np.any(bv))

    nc = _build_program(use_bias_qk, use_bias_v)
    LAST_NC = nc

    xT = {}
    for b in range(B):
        xT[("q", b)] = _f16(q[b].T)
        xT[("k", b)] = _f16(k[b].T)
        xT[("v", b)] = _f16(v[b].T)
        xT[("m", b)] = np.ascontiguousarray(
            mask[b, 0].T.astype(np.float32)).astype(ml_dtypes.float8_e4m3)

    in_maps = []
    for c in range(8):
        b, g = divmod(c, G)
        js = slice(g * J, (g + 1) * J)
        in_maps.append({
            "xqT": xT[("q", b)],
            "xkT": xT[("k", b)],
            "xvT": xT[("v", b)],
            "maskT": xT[("m", b)],
            "wqT": _f16(Wq[js, :].T),
            "wkT": _f16(Wk[js, :].T),
            "wvT": _f16(Wv[js, :].T),
            "woT": _f16(Wo[:, js].T),
            "bqk": np.ascontiguousarray(
                np.stack([bq[js], bk[js]]).astype(np.float32)),
            "bv": np.ascontiguousarray(bv[js].reshape(1, J)),
        })

    os.environ["BASS_NEVER_TRACE"] = "1"
    res = bass_utils.run_bass_kernel_spmd(
        nc, in_maps, core_ids=list(range(8)), trace=False,
    )
    LAST_RESULTS = res

    full = np.zeros((B, S, E), np.float32)
    for c in range(8):
        b = c // G
        full[b] += res.results[c]["out"].astype(np.float32)
    full += bo[None, None, :]
    return full
